# revision 27
# baseline (speedup 1.0000x reference)
"""Trainium2 Bass kernel for CSNet14: 14-layer tiny MLP over 2M x 12 batch.

Strategy v3.1 (pure data parallel, 8 cores; ~230.4us/core cost-model
time, steady state PE ~94% busy = the binder at 20 matmuls/round):
  - Shard x along batch: 250000 samples/core (padded to 250880 = 49*5120).
  - Host packs x feature-major with a constant-ones row 120:
    xt[12g+i, c] = x[g*25088 + c, i]; xt[120, c] = 1.  The [121, 512]
    matmul operand tiles stream straight from DRAM (2 KB descriptors),
    so there are no on-chip transposes at all.
  - Each linear layer = one PE matmul with a host-built block-diagonal
    lhsT [121,121] (10 copies of W_l^T, bias row 120, corner 1 so the
    ones row self-propagates through relu).  bfloat16 operands (rel err
    3.2e-3, well under the 2e-2 gate), 512-col tiles -> 1 cycle/row.
  - Biases ride the matmul, so epilogues are a single pure-relu op per
    layer, alternated between ScalarE activation(Relu) and VectorE
    tensor_scalar(max) (KERNEL_EPI; GpSimd cannot read PSUM).
  - Decoder skip-adds: PE accumulating identity matmul into the same
    PSUM bank (KERNEL_SKIP can move some to DVE tensor_tensor).
  - fc14 + softmax folded to d = (W14[0]-W14[1])h + (b14[0]-b14[1]) with
    the bias via the ones row; d-rows for 12 rounds accumulate into one
    PSUM bank (rows 30c+10s+g), then p0 = sigmoid(d), p1 = sigmoid(-d)
    written interleaved; g-major output rows make each round's store DMA
    4 KB-contiguous per group row.
  - Diagonal software pipeline: issue slot s covers layer l = s - r for
    all in-flight rounds r, so each slot's engine demand is the full
    per-round mix and no slot has internal dependency chains.  Steady
    state runs PE ~99% / DVE ~99% / ACT ~95% busy.
  - Weights load in two 512 B-row DMA groups ordered so fc1 is ready
    ~1us in; the last round is split into half-width specs and the final
    full superround drains in 5-round output groups, so sigmoid+store
    overlap the pipeline drain (KERNEL_EDGE/KERNEL_TAILQ).
  - Tuned: KERNEL_DT=bf16, KERNEL_H2BUFS=7 (h13 ring depth was a WAR
    stall), KERNEL_TAILQ=5.  Explored and rejected (all slower on the
    cost model): FD-1024 paired epilogues (PSUM 8-bank ring couples PE to
    ACT/DVE same-slot), DEC_G GpSimd relu offload (3-engine chain
    latency), fc14 round-pair matmuls (PSS=6 ring regression), fp8
    DoubleRow (rhs K-interleave breaks the chained-layer tile layout).
"""

import os
import sys
from contextlib import ExitStack

import numpy as np

for _p in ("/opt/trn_rl_repo", "/root/.axon_site/_ro/trn_rl_repo"):
    if os.path.isdir(_p) and _p not in sys.path:
        sys.path.append(_p)

import concourse.bass as bass
import concourse.bacc as bacc
import concourse.mybir as mybir
import concourse.tile as tile
from concourse.bass_utils import run_bass_kernel_spmd

DIMS = [(12, 12), (12, 11), (11, 10), (10, 9), (9, 8), (8, 7), (7, 6),
        (6, 7), (7, 8), (8, 9), (9, 10), (10, 11), (11, 12), (12, 2)]
BATCH = 2_000_000
NCORES = 8
G = 10                 # sample-groups (12-row stride in the 120-row tiles)
NT = 512               # free-dim columns per round (one PSUM bank)
SPR = G * NT           # samples per round = 5120
B_CORE = BATCH // NCORES
ROUNDS = -(-B_CORE // SPR)          # 49
B_PAD = ROUNDS * SPR                # 250880
B10 = B_PAD // G                    # columns per group-row = 25088
RP = 121               # tile rows: 120 data + ones row

F32 = mybir.dt.float32
F32R = mybir.dt.float32r

DECODER = range(7, 13)  # encoder layers 0..5 produce ids consumed here

KDT = os.environ.get("KERNEL_DT", "bf16")
MM_DT = {"bf16": mybir.dt.bfloat16, "f32": F32, "f32r": F32R}[KDT]
X_DT = {"bf16": mybir.dt.bfloat16, "f32": F32, "f32r": F32R}[KDT]

# epilogue engine mix (indexed by pair/single slot position rotated by s):
# a=ScalarE(activation Relu) d=VectorE p=GpSimd (tensor_scalar max 0)
EPI = os.environ.get("KERNEL_EPI", "ad")
# decoder layers whose skip-add+relu run as DVE tensor_tensor (psum+id->t
# in SBUF) followed by a GpSimd relu (t->h): frees one PE matmul and one
# ACT/DVE relu per layer per round, at the cost of a DVE add + Pool relu
_dg = os.environ.get("KERNEL_DECG", "")
DEC_G = tuple(int(c) for c in _dg.split(",")) if _dg else ()
# decoder skip-add engine by round: p=PE matmul, d=DVE tensor_tensor
SKIP = os.environ.get("KERNEL_SKIP", "p")
LOOKAHEAD = int(os.environ.get("KERNEL_LOOKAHEAD", "4"))
NPAIR = int(os.environ.get("KERNEL_NPAIR", "0"))   # epi pairs per slot (0..6)
# fc14 round-pairing: steady superrounds issue one N=1024 wd matmul per
# round-PAIR (rhs = adjacent h13 halves), halving fc14's PE cost; the psd
# tile widens to 2 banks (PSS drops to 6).  wd block p's rows are 10p+g,
# which is exactly the row set a round-pair shares, so the same wd blocks
# serve both the paired steady path and the per-round tail path.
FC14PAIR = os.environ.get("KERNEL_FC14PAIR", "0") == "1"


def build_nc(rounds=ROUNDS):
    """Build and compile the single-core Bass program (run SPMD on 8 cores)."""
    nc = bacc.Bacc("TRN2", target_bir_lowering=False, debug=False)

    x_t = nc.dram_tensor("x", [RP, B10], X_DT, kind="ExternalInput")
    y_t = nc.dram_tensor("y", [B_PAD, 2], F32, kind="ExternalOutput")
    wblk_t = nc.dram_tensor("wblk", [13, RP, 128], MM_DT, kind="ExternalInput")
    wid_t = nc.dram_tensor("wid", [RP, 128], MM_DT, kind="ExternalInput")
    wd_t = nc.dram_tensor("wd", [12, RP, 128], MM_DT, kind="ExternalInput")

    x = x_t.ap()
    # [G, B10, 2] view: row g holds samples g*B10 .. (g+1)*B10-1
    yv = y_t.ap().rearrange("(g n) c -> g n c", g=G)
    yr = y_t.ap().rearrange("(g r n) c -> g r n c", g=G, n=NT)

    with tile.TileContext(nc) as tc, ExitStack() as ctx:
        const = ctx.enter_context(tc.tile_pool(name="const", bufs=1))
        xin = ctx.enter_context(tc.tile_pool(name="xin", bufs=int(os.environ.get("KERNEL_XBUFS", "6"))))
        # pair short-lived layers first: long-lived id tiles are cheaper
        # as singles (pair tiles are 2x wide)
        PAIRS = [(10, 11), (8, 9), (6, 7), (4, 5), (2, 3), (0, 1)][:NPAIR]
        PAIRED = {l for p in PAIRS for l in p}
        # id tile h_k is consumed 14-2k diagonal slots after it is written;
        # singles of paired layers only occur at pipeline boundaries
        IDX = int(os.environ.get("KERNEL_IDEXTRA", "0"))
        idp = {k: ctx.enter_context(tc.tile_pool(
                   name=f"id{k}",
                   bufs=2 if (k - 1) in PAIRED else 15 - 2 * k + IDX))
               for k in range(1, 7)}
        hp2 = ctx.enter_context(tc.tile_pool(name="h2", bufs=int(os.environ.get("KERNEL_H2BUFS", "7"))))
        hp2b = ctx.enter_context(tc.tile_pool(name="h2b", bufs=2))
        tpool = ctx.enter_context(tc.tile_pool(
            name="tp", bufs=int(os.environ.get("KERNEL_TBUFS", "4")))) if DEC_G else None
        yp = ctx.enter_context(tc.tile_pool(name="y", bufs=2))
        pspair = ctx.enter_context(
            tc.tile_pool(name="pspair", bufs=int(os.environ.get("KERNEL_PSP", "1")), space=bass.MemorySpace.PSUM)) if NPAIR else None
        pssing = ctx.enter_context(
            tc.tile_pool(name="pssing", bufs=int(os.environ.get(
                "KERNEL_PSS",
                ("6" if FC14PAIR else "7") if not NPAIR else "3")),
                space=bass.MemorySpace.PSUM))
        h13p = ctx.enter_context(tc.tile_pool(name="h13p", bufs=3)) if FC14PAIR else None
        psd_pool = ctx.enter_context(
            tc.tile_pool(name="psd", bufs=int(os.environ.get("KERNEL_PSD", "1")), space=bass.MemorySpace.PSUM))

        # ---- constants: per-layer 512B-row DMAs so layer 0 is ready
        # almost immediately and the pipeline fill overlaps the rest ----
        w_sb = const.tile([RP, 13 * 128], MM_DT, tag="wsb")
        wid_sb0 = const.tile([RP, 128], MM_DT, tag="wid")
        wid_sb = wid_sb0[:, 0:RP]
        wd_sb = const.tile([RP, 12 * 128], MM_DT, tag="wd")

        def load_w(lo, hi):
            nc.sync.dma_start(
                w_sb[:, lo * 128:hi * 128].rearrange(
                    "k (l m) -> k l m", l=hi - lo, m=128),
                wblk_t.ap()[lo:hi].rearrange("l k m -> k l m"))

        def load_consts_late():
            nc.sync.dma_start(wid_sb0[:], wid_t.ap())
            nc.sync.dma_start(
                wd_sb[:].rearrange("k (q m) -> k q m", q=12, m=128),
                wd_t.ap().rearrange("q k m -> k q m"))

        sup = None
        # edge rounds split into half-width specs: halves the per-layer
        # latency during pipeline fill/drain
        EDGE = int(os.environ.get("KERNEL_EDGE", "1"))
        H = NT // 2
        SPECS = []
        EDGEH = int(os.environ.get("KERNEL_EDGEH", "0"))
        for r in range(rounds):
            if r < EDGEH or r >= rounds - EDGE:
                SPECS.append((r, 0, H))
                SPECS.append((r, H, H))
            else:
                SPECS.append((r, 0, NT))
        NV = len(SPECS)
        hs, idm = {}, {v: {} for v in range(NV)}

        def issue_mm(v, l, out_ap):
            r = SPECS[v][0]
            if l in DEC_G:
                # g-mode: skip-add happens later as a DVE tensor_tensor
                nc.tensor.matmul(
                    out_ap, w_sb[:, l * 128:l * 128 + RP], hs[v],
                    start=True, stop=True)
                return
            skip_eng = SKIP[r % len(SKIP)] if l in DECODER else None
            nc.tensor.matmul(
                out_ap, w_sb[:, l * 128:l * 128 + RP], hs[v],
                start=True, stop=(skip_eng != "p"))
            if l in DECODER:
                idh = idm[v].pop(12 - l)
                if skip_eng == "p":
                    nc.tensor.matmul(out_ap, wid_sb, idh,
                                     start=False, stop=True)
                else:
                    nc.vector.tensor_tensor(
                        out_ap[0:120, :], out_ap[0:120, :], idh[0:120, :],
                        mybir.AluOpType.add)

        h13pair = {}   # even round -> its [RP, 2NT] pair tile

        def alloc_h(v, l):
            w = SPECS[v][2]
            if FC14PAIR and l == 12:
                r = SPECS[v][0]
                if r < LF:   # steady superrounds: adjacent h13 halves
                    if r % 2 == 0:
                        hp = h13p.tile([RP, 2 * NT], MM_DT, tag="h13p",
                                       name=f"h13p_{r}")
                        h13pair[r] = hp
                        hs[v] = hp[:, 0:w]
                        return hp
                    hp = h13pair[r - 1]
                    hs[v] = hp[:, NT:NT + w]
                    return hp
            pool_l = (idp[l + 1] if l < 6 else
                      (hp2b if l in PAIRED else hp2))
            h2 = pool_l.tile([RP, NT], MM_DT, tag=f"h{l + 1}",
                             name=f"h{l + 1}_{v}")
            if l < 6:
                idm[v][l] = h2[:, 0:w]
            hs[v] = h2[:, 0:w]
            return h2

        def issue_epi(eng, out_ap, ps_ap):
            if eng == "a":
                nc.scalar.activation(out_ap, ps_ap,
                                     mybir.ActivationFunctionType.Relu)
            elif eng == "d":
                nc.vector.tensor_scalar_max(out_ap, ps_ap, 0.0)
            else:
                nc.gpsimd.tensor_scalar_max(out_ap, ps_ap, 0.0)

        # fixed layer-pairs sharing one [RP, 2NT] psum tile + one epilogue;
        # hpools[j] holds the paired h tiles, rings sized to id lifetimes
        def lifet(l):  # slots between write of h_{l+1} and its last read
            k = l + 1
            return 14 - 2 * k if k <= 6 else 1

        hpair = {j: ctx.enter_context(
            tc.tile_pool(name=f"hp{j}",
                         bufs=max(lifet(l0), lifet(l1)) + 2))
                 for j, (l0, l1) in enumerate(PAIRS)}

        def reg_h(r, l, ap):
            if l < 6:
                idm[r][l] = ap
            hs[r] = ap

        def issue_single(s, v, l, k):
            w = SPECS[v][2]
            ps = pssing.tile([RP, NT], F32, tag="ps", name=f"ps_{s}_{k}")
            issue_mm(v, l, ps[:, 0:w])
            h2 = alloc_h(v, l)
            if l in DEC_G:
                idh = idm[v].pop(12 - l)
                t = tpool.tile([RP, NT], MM_DT, tag="tp", name=f"tp_{s}_{k}")
                nc.vector.tensor_tensor(t[:, 0:w], ps[:, 0:w], idh,
                                        mybir.AluOpType.add)
                nc.gpsimd.tensor_scalar_max(h2[:, 0:w], t[:, 0:w], 0.0)
            else:
                issue_epi(EPI[(s + k) % len(EPI)], h2[:, 0:w], ps[:, 0:w])

        # output groups: 12-round superrounds in steady state; the tail
        # (last full superround onward) drains in 3-round groups so the
        # sigmoid+store overlap the pipeline drain instead of trailing it
        K_LAST_FULL = rounds // 12 - 1 if rounds >= 12 else 0
        LF = 12 * K_LAST_FULL
        TAILQ = int(os.environ.get("KERNEL_TAILQ", "5"))

        def grp(r2):
            return r2 // 12 if r2 < LF else 1000 + (r2 - LF) // TAILQ

        gspecs = {}
        for i, (r2, c2, w2) in enumerate(SPECS):
            gspecs.setdefault(grp(r2), []).append(i)
        glast = {}
        for g2, vs in gspecs.items():
            for i in vs:
                glast[(g2, SPECS[i][1])] = i

        PSDW = 2 * NT if FC14PAIR else NT

        def issue_fc14(v):
            # d-rows of round r land in psd rows 10*rr + g (full-AP
            # accumulate; each spec owns its own column range).  In
            # FC14PAIR mode the steady superrounds issue one N=1024 matmul
            # per round-pair: wd block p's rows (10p+g) are shared by
            # rounds 2p (cols 0:NT) and 2p+1 (cols NT:2NT).
            nonlocal sup
            r, c0, w = SPECS[v]
            rr = r % 12
            g2 = grp(r)
            if sup is None:
                psd_tile = psd_pool.tile([128, PSDW], F32, tag="psd",
                                         name=f"psd_{r}")
                sup = [psd_tile, [], len(gspecs[g2]), set()]
            psd = sup[0]
            tail = g2 >= 1000
            # tail groups reuse the first wd blocks, whose rows are 10s+g,
            # so the sigmoid read starts at partition 0 (alignment rule)
            blk = (r - LF) % TAILQ if tail else rr
            pair_steady = FC14PAIR and not tail
            if pair_steady:
                if rr % 2 == 1:
                    p = rr // 2
                    nc.tensor.matmul(psd[:, 0:2 * NT],
                                     wd_sb[:, p * 128:(p + 1) * 128],
                                     h13pair.pop(r - 1)[:, 0:2 * NT],
                                     start=(p == 0), stop=(p == 5),
                                     skip_group_check=True)
            else:
                gran = range(c0 // H, (c0 + w) // H)
                nc.tensor.matmul(psd[:, c0:c0 + w],
                                 wd_sb[:, blk * 128:(blk + 1) * 128],
                                 hs[v],
                                 start=not any(i in sup[3] for i in gran),
                                 stop=(glast[(g2, c0)] == v),
                                 skip_group_check=True)
                sup[3].update(gran)
            if r not in sup[1]:
                sup[1].append(r)
            sup[2] -= 1

            if sup[2] == 0:
                def rowof(r2):
                    if tail:
                        return 10 * ((r2 - LF) % TAILQ)
                    if pair_steady:
                        return 10 * ((r2 % 12) // 2)
                    return 30 * ((r2 % 12) // 3) + 10 * (r2 % 3)
                nrow = max(rowof(r2) for r2 in sup[1]) + 10
                sw = 2 * NT if pair_steady else NT
                yt = yp.tile([128, 2 * PSDW], F32, tag="yt")
                nc.scalar.activation(
                    yt[0:nrow, 0:2 * sw:2], psd[0:nrow, 0:sw],
                    mybir.ActivationFunctionType.Sigmoid, scale=1.0)
                nc.scalar.activation(
                    yt[0:nrow, 1:2 * sw:2], psd[0:nrow, 0:sw],
                    mybir.ActivationFunctionType.Sigmoid, scale=-1.0)
                for r2 in sup[1]:
                    row = rowof(r2)
                    cofs = 2 * NT * (r2 % 2) if pair_steady else 0
                    nc.sync.dma_start(
                        yv[:, r2 * NT:(r2 + 1) * NT, :],
                        yt[row:row + G, cofs:cofs + 2 * NT].rearrange(
                            "g (n c) -> g n c", c=2))
                sup = None

        def issue_load(v):
            r, c0, w = SPECS[v]
            xt = xin.tile([RP, NT], X_DT, tag="xt", name=f"xt_{v}")
            nc.sync.dma_start(xt[:, 0:w], x[:, r * NT + c0:r * NT + c0 + w])
            hs[v] = xt[:, 0:w]

        NL = 13
        issue_load(0)
        load_w(0, 2)
        issue_load(1)
        load_w(2, 13)
        for v in range(2, min(LOOKAHEAD + 1, NV)):
            issue_load(v)
        load_consts_late()
        # g-mode layers issue first within the slot so their 3-engine
        # chain (MM -> DVE add -> Pool relu) starts at the head of the DVE
        # and Pool queues; their outputs are needed early next slot
        GFIRST = os.environ.get("KERNEL_GFIRST", "1") == "1"
        for s in range(NV + NL - 1):
            lv = [(s - v, v) for v in range(min(NV - 1, s), max(-1, s - NL), -1)]
            if DEC_G and GFIRST:
                lv.sort(key=lambda t: (0 if t[0] in DEC_G else 1, t[0]))
            k = 0
            for l, v in lv:
                issue_single(s, v, l, k)
                k += 1
            nxt = s + LOOKAHEAD + 1
            if nxt < NV:
                issue_load(nxt)
            rf = s - NL + 1
            if 0 <= rf < NV:
                issue_fc14(rf)

    nc.compile()
    return nc


def host_prep(inputs):
    """Build the block-diagonal weight/bias blobs from the raw params."""
    Ws = [np.asarray(inputs[f"w{i + 1}"], np.float32) for i in range(14)]
    Bs = [np.asarray(inputs[f"b{i + 1}"], np.float32) for i in range(14)]

    wblk = np.zeros((13, RP, 128), np.float32)
    for l in range(13):
        din, dout = DIMS[l]
        for g in range(G):
            wblk[l, 12 * g:12 * g + din, 12 * g:12 * g + dout] = Ws[l].T
            wblk[l, 120, 12 * g:12 * g + dout] = Bs[l]
        # ones row self-propagates via the corner 1; for g-mode decoder
        # layers the ones come from the id tensor through the DVE add, so
        # zero the corner there to avoid doubling
        wblk[l, 120, 120] = 0.0 if l in DEC_G else 1.0
    wid = np.zeros((RP, 128), np.float32)
    wid[:120, :120] = np.eye(120, dtype=np.float32)  # no ones-row double-add
    wd = np.zeros((12, RP, 128), np.float32)
    wdvec = Ws[13][0] - Ws[13][1]          # [12]
    bd = float(Bs[13][0] - Bs[13][1])
    for rr in range(12):
        row = 30 * (rr // 3) + 10 * (rr % 3)
        for g in range(G):
            wd[rr, 12 * g:12 * g + 12, row + g] = wdvec
            wd[rr, 120, row + g] = bd      # fc14 bias via ones row
    if KDT == "bf16":
        import ml_dtypes
        bf = ml_dtypes.bfloat16
        wblk, wid, wd = (a.astype(bf) for a in (wblk, wid, wd))
    return dict(wblk=wblk, wid=wid, wd=wd)


_NC_CACHE = {}


def _get_nc():
    key = (ROUNDS, KDT, EPI, SKIP, NPAIR, LOOKAHEAD, DEC_G, FC14PAIR)
    if key not in _NC_CACHE:
        _NC_CACHE[key] = build_nc()
    return _NC_CACHE[key]


def kernel(**inputs):
    x = np.ascontiguousarray(np.asarray(inputs["x"], np.float32))
    consts = host_prep(inputs)

    # per-core feature-major pack + ones row:
    # xt[12g+i, c] = xcore[g*B10 + c, i]; xt[120, :] = 1
    xpad = np.zeros((NCORES, B_PAD, 12), np.float32)
    xpad[:, :B_CORE] = x.reshape(NCORES, B_CORE, 12)
    xt = np.empty((NCORES, RP, B10), np.float32)
    xt[:, :120] = np.ascontiguousarray(
        xpad.reshape(NCORES, G, B10, 12).transpose(0, 1, 3, 2)
    ).reshape(NCORES, 120, B10)
    xt[:, 120] = 1.0

    if KDT == "bf16":
        import ml_dtypes
        xt = xt.astype(ml_dtypes.bfloat16)
    in_maps = [dict(x=xt[c], **consts) for c in range(NCORES)]
    nc = _get_nc()
    res = run_bass_kernel_spmd(
        nc, in_maps, core_ids=list(range(NCORES)),
        trace=os.environ.get("KERNEL_TRACE", "0") == "1")
    kernel.last_results = res
    # rows are already in original order: sample_id = g*B10 + c
    y = np.concatenate([res.results[c]["y"][:B_CORE] for c in range(NCORES)],
                       axis=0)
    return y


if __name__ == "__main__":
    nc = build_nc()
    print("compiled OK")

